# revision 4
# baseline (speedup 1.0000x reference)
"""Trainium2 Bass kernel for topk_masking row-parallel linear.

Reference semantics:
    idx  = argmax_k(score[o, i, :])            (first index wins ties)
    net  = weight[o, i, idx]                   [OUT, IN]
    out  = x @ net.T                           [BATCH, OUT]

Device algorithm (per core, o-shard of 256 out-features, exact):
    data layout [i, (o, k)]  (host pre-transposed; i on partitions)
    m   = segmented_max_k(s)                   1 DVE reduce pass
    t   = s - m                                (<= 0, == 0 only at argmax)
    v   = t * BIG + w                          (= w at argmax, < -2*std elsewhere)
    net = segmented_max_k(v)                   -> net in [i, o] layout == net.T
    outT[o, b] += net.T[i, o] chunks @ xT[i, b] on the PE, accumulated in PSUM

BIG * (minimum nonzero |s_i - s_j|) >> 2*std, so the penalized candidates can
never beat the argmax one; ties at the segment max do not occur in this input
distribution (verified: fp32 scores within a slot are distinct).
"""

import sys

import numpy as np

if "/opt/trn_rl_repo" not in sys.path:
    sys.path.insert(0, "/opt/trn_rl_repo")

import concourse.bacc as bacc
import concourse.tile as tile
from concourse import mybir
from concourse.bass_utils import run_bass_kernel_spmd

OUT_F, IN_F, K, BATCH = 2048, 2048, 8, 256
N_CORES = 8
OSH = OUT_F // N_CORES  # 256 out-features per core
P = 128
NBLK = IN_F // P        # 16 contraction blocks
FREE = OSH * K          # 2048 f32 per partition row of a w/s shard block
BIG = 1e10
F32 = mybir.dt.float32
AX_X = mybir.AxisListType.X
ALU = mybir.AluOpType

# Engine split: o-columns [0, O_SUB_DVE) of the subtract pass run on the DVE,
# the rest on GPSIMD; same for the scalar_tensor_tensor (mask+select) pass.
O_SUB_DVE = 90
O_STT_DVE = 90


def build(o_sub_dve=O_SUB_DVE, o_stt_dve=O_STT_DVE, io_bufs=3, mid_bufs=2):
    nc = bacc.Bacc("TRN2", target_bir_lowering=False, debug=False)
    w_d = nc.dram_tensor("w", [IN_F, FREE], F32, kind="ExternalInput")
    s_d = nc.dram_tensor("s", [IN_F, FREE], F32, kind="ExternalInput")
    x_d = nc.dram_tensor("xt", [IN_F, BATCH], F32, kind="ExternalInput")
    o_d = nc.dram_tensor("outT", [OSH, BATCH], F32, kind="ExternalOutput")

    w_blk = w_d.ap().rearrange("(n p) f -> n p f", p=P)
    s_blk = s_d.ap().rearrange("(n p) f -> n p f", p=P)
    x_blk = x_d.ap().rearrange("(n p) b -> p n b", p=P)
    o_blk = o_d.ap().rearrange("(h p) b -> h p b", p=P)

    with tile.TileContext(nc) as tc:
        with (
            tc.tile_pool(name="io", bufs=io_bufs) as io,
            tc.tile_pool(name="mid", bufs=mid_bufs) as mid,
            tc.tile_pool(name="small", bufs=mid_bufs) as small,
            tc.tile_pool(name="stat", bufs=1) as stat,
            tc.tile_pool(name="ps", bufs=1, space="PSUM") as psp,
        ):
            xt_sb = stat.tile([P, NBLK * BATCH], F32)
            xt3 = xt_sb[:].rearrange("p (n b) -> p n b", b=BATCH)
            nc.sync.dma_start(xt3, x_blk)

            ps0 = psp.tile([P, BATCH], F32)
            ps1 = psp.tile([P, BATCH], F32)

            for n in range(NBLK):
                w_sb = io.tile([P, FREE], F32)
                s_sb = io.tile([P, FREE], F32)
                nc.sync.dma_start(s_sb[:], s_blk[n])
                nc.sync.dma_start(w_sb[:], w_blk[n])

                s3 = s_sb[:].rearrange("p (o k) -> p o k", k=K)

                m = small.tile([P, OSH], F32)
                nc.vector.reduce_max(m[:], s3, axis=AX_X)
                mb = m[:].unsqueeze(2).broadcast_to([P, OSH, K])

                t_sb = mid.tile([P, FREE], F32)
                t3 = t_sb[:].rearrange("p (o k) -> p o k", k=K)
                c = o_sub_dve
                if c > 0:
                    nc.vector.tensor_tensor(
                        t3[:, :c, :], s3[:, :c, :], mb[:, :c, :], ALU.subtract
                    )
                if c < OSH:
                    nc.gpsimd.tensor_tensor(
                        t3[:, c:, :], s3[:, c:, :], mb[:, c:, :], ALU.subtract
                    )

                # w is pre-scaled by 2^-34 on the host, so t + w == (s-m) + w/BIG
                # exactly; the segmented max of this selects the argmax-k weight
                # (scaled), and x is pre-scaled by 2^34 to cancel in the matmul.
                v_sb = mid.tile([P, FREE], F32)
                d = o_stt_dve * K
                if d > 0:
                    nc.vector.tensor_tensor(
                        v_sb[:, :d], t_sb[:, :d], w_sb[:, :d], ALU.add
                    )
                if d < FREE:
                    nc.gpsimd.tensor_tensor(
                        v_sb[:, d:], t_sb[:, d:], w_sb[:, d:], ALU.add
                    )

                net = small.tile([P, OSH], F32)
                v3 = v_sb[:].rearrange("p (o k) -> p o k", k=K)
                nc.vector.reduce_max(net[:], v3, axis=AX_X)

                nc.tensor.matmul(
                    ps0[:], net[:, 0:P], xt3[:, n, :],
                    start=(n == 0), stop=(n == NBLK - 1),
                )
                nc.tensor.matmul(
                    ps1[:], net[:, P:OSH], xt3[:, n, :],
                    start=(n == 0), stop=(n == NBLK - 1),
                )

            ob0 = stat.tile([P, BATCH], F32)
            ob1 = stat.tile([P, BATCH], F32)
            nc.scalar.copy(ob0[:], ps0[:])
            nc.scalar.copy(ob1[:], ps1[:])
            nc.sync.dma_start(o_blk[0], ob0[:])
            nc.sync.dma_start(o_blk[1], ob1[:])
    nc.compile()
    return nc


def make_in_maps(x, weight, score):
    # Exact power-of-2 pre-scaling: w' = w * 2^-34, x' = x * 2^34. The device
    # computes net' = net * 2^-34 and out = x' @ net'.T == x @ net.T exactly.
    w_scaled = np.asarray(weight, dtype=np.float32) * np.float32(2.0**-34)
    x_scaled = np.asarray(x, dtype=np.float32) * np.float32(2.0**34)
    w_t = np.transpose(w_scaled, (1, 0, 2))                              # [IN, OUT, K]
    s_t = np.transpose(np.asarray(score, dtype=np.float32), (1, 0, 2))
    xt = np.ascontiguousarray(x_scaled.T)                                # [IN, BATCH]
    in_maps = []
    for c in range(N_CORES):
        sl = slice(c * OSH, (c + 1) * OSH)
        in_maps.append(
            {
                "w": np.ascontiguousarray(w_t[:, sl, :]).reshape(IN_F, FREE),
                "s": np.ascontiguousarray(s_t[:, sl, :]).reshape(IN_F, FREE),
                "xt": xt,
            }
        )
    return in_maps


def assemble_out(results):
    outT = np.concatenate([results[c]["outT"] for c in range(N_CORES)], axis=0)
    return np.ascontiguousarray(outT.T)  # [BATCH, OUT]


def run(x, weight, score, trace=False, nc=None):
    """Returns (out, BassKernelResults)."""
    if nc is None:
        nc = build()
    res = run_bass_kernel_spmd(
        nc, make_in_maps(x, weight, score), list(range(N_CORES)), trace=trace
    )
    return assemble_out(res.results), res


def kernel(x, weight, score):
    out, _ = run(x, weight, score, trace=False)
    return out
